# revision 23
# baseline (speedup 1.0000x reference)
"""AttentionHead kernel for 8 TRN2 NeuronCores — v5.7 (streaming + fp8 DR).

Reference computation (B=4, S=2048, D=1024, dk=dv=64):
    q = query @ Wq + bq ; k = key @ Wk + bk ; v = value @ Wv + bv
    out = softmax(q @ k.T / 8) @ v

Sharding: core i handles batch b = i//2 and KEY/VALUE half kvh = i%2:
it attends ALL 2048 queries of its batch against its 1024 keys, producing
a partial softmax numerator [64, 2048] and denominator [1, 2048]. Since
max-subtraction is skipped (scores std ~0.33), the host combines halves
by adding numerators and denominators, then divides — zero d2d traffic.

Key design points (lineage 149 -> 98/67 -> 61 -> this):
  * Tile-major activation layouts ([tile, P, DC, 512] blocks): each tile
    is one 128-descriptor HWDGE DMA. fp8 q/k ride RAW (the PE accepts
    fp8 moving operands); no gpsimd SWDGE anywhere.
  * ALL DMAs (consts + inputs) ride ONE HWDGE ring (sync) in
    consumption order: same-ring transfers are strictly FIFO at full
    aggregate bandwidth (measured), so tensors complete in stream order
    (q0, k0, k1, v0, v1, q1..q3) with zero chaining overhead. A second
    active ring would interleave and wreck the order; completion-
    chaining via deps costs 2-3us dead time per link. Both measured
    worse.
  * q and k projections run as fp8 DoubleRow matmuls (2 D-chunks per
    instruction, 2x): weights ship as fp8 scaled by 16 (entries +-0.5,
    inside e4m3 normal range; unscaled they'd be subnormal), activations
    are the raw fp8 inputs. The resulting 256x score scale is folded
    into the exp: ACT Exp uses scale=1/2048, the Schraudolph multiplier
    absorbs the same factor. 16*bq rides as bias on the qt copy.
  * v projection stays bf16 (quantizing v costs ~2.4% end-to-end).
  * Scores are row-tiled: kt2 holds two 64-contraction strips (key
    chunks on partitions 0:64 / 64:128); two K=64 matmuls at
    tile_position (0,0)/(64,0) overlap on the PE. The q projection
    lands duplicated on both partition halves via host-duplicated Wq
    columns (free: output partitions don't cost cycles).
  * attn@v is software-pipelined one slot behind scores so the PE never
    stalls on exp latency; exp splits ACT (exact, scaled Exp) / DVE
    (Schraudolph: i16 = round(x*A + 16248.5) bitcast bf16 ~ e^x).
    q_proj(t+1) is emitted into tile t's attn@v tail (fills the PE
    bubble); an 8-MM single-accumulation warmup locks HAM to K=8/8
    before the first projection. PSUM: sc x4 + kv x2 (k rows 0:64 via
    DoubleRow, v rows 64:128 via col offset, pt shares) + psQ x1 +
    po x1 = 8 banks.
  * Softmax denominator via a ones-column appended to v (row 64 of po).
  * Per-tile output copy (split ACT/DVE halves) + per-tile DMA.
"""

import os
import sys

if "/opt/trn_rl_repo" not in sys.path:
    sys.path.insert(0, "/opt/trn_rl_repo")

import numpy as np
import ml_dtypes

import concourse.bass as bass
import concourse.mybir as mybir
import concourse.tile as tile
from concourse import bacc
from concourse.bass_utils import run_bass_kernel_spmd
from concourse.masks import make_identity
from concourse.tile_rust import add_dep_helper

N_CORES = 8
B, S, D, DK = 4, 2048, 1024, 64
S_KV = S // 2           # per-core key/value rows
P = 128
DC = D // P             # 8 contraction chunks
QT = 512                # tile width (PSUM bank: 512 f32)
NQT = S // QT           # 4 query tiles
NKV = S_KV // QT        # 2 kv tiles
CPT = QT // P           # key chunks per kv tile (4)
VW = DK + 1             # v plus ones-column
VPAD = 66               # v_pack row stride (VW padded to 4B multiple)
F32 = mybir.dt.float32
BF16 = mybir.dt.bfloat16
F8 = mybir.dt.float8e4
I16 = mybir.dt.int16
BF = ml_dtypes.bfloat16
F8NP = ml_dtypes.float8_e4m3
DR = mybir.MatmulPerfMode.DoubleRow

# Schraudolph exp for bf16: bitcast(int16(round(x * 128/ln2 + b))) ~ e^x
SCHRAU_A = 128.0 * 1.4426950408889634
SCHRAU_B = 16248.5
WSCALE = 16.0           # fp8 weight scale (q/k); descaled inside exp
ESCALE = 1.0 / (8.0 * WSCALE * WSCALE)   # exp(scores_hw * ESCALE)

N_WARMUP = int(os.environ.get("BASS_ATTN_WARMUP", "11"))
# exp engine per key-chunk slot, cycled: a=ACT (exact), v=DVE (Schraudolph)
EXP_PATTERN = os.environ.get("BASS_ATTN_EXP_PATTERN", "avavaavv")
USE_DR = os.environ.get("BASS_ATTN_DR", "1") == "1"   # fp8 DoubleRow q/k proj
CHAIN_DEPTH = int(os.environ.get("BASS_ATTN_CHAIN", "3"))


def build_program(n_warmup=N_WARMUP, exp_pattern=EXP_PATTERN, use_dr=USE_DR):
    nc = bacc.Bacc("TRN2", target_bir_lowering=False, debug=False,
                   num_devices=N_CORES)

    qT_d = nc.dram_tensor("qT", [NQT, P, DC, QT], F8, kind="ExternalInput")
    kT_d = nc.dram_tensor("kT", [NKV, P, DC, QT], F8, kind="ExternalInput")
    vT_d = nc.dram_tensor("vT", [NKV, P, DC, QT], BF16, kind="ExternalInput")
    # Wall (bf16 path): [P, DC, 4, DK] = [Wq | Wq | Wk | Wv]
    # W8 (DoubleRow path): [P, DC, 3, DK] fp8 = [16Wq | 16Wq | 16Wk]
    wall_d = nc.dram_tensor("Wall", [P, DC, 4, DK], BF16,
                            kind="ExternalInput")
    w8_d = nc.dram_tensor("W8", [P, DC, 3, DK], F8, kind="ExternalInput")
    wv_d = nc.dram_tensor("Wv16", [P, DC, DK], BF16, kind="ExternalInput")
    bqd_d = nc.dram_tensor("bqd", [P, 1], F32, kind="ExternalInput")
    # rows 0:64 = partial attn@v numerator, row 64 = partial softmax
    # denominator; the host combines kv-halves, divides, adds bv.
    out_d = nc.dram_tensor("out", [VW, S], BF16, kind="ExternalOutput")

    from contextlib import ExitStack

    with tile.TileContext(nc) as tc, ExitStack() as ctx:
        consts = ctx.enter_context(tc.tile_pool(name="consts", bufs=1))
        kvp = ctx.enter_context(tc.tile_pool(name="kvp", bufs=2))
        qp = ctx.enter_context(tc.tile_pool(name="qp", bufs=4))
        sbuf = ctx.enter_context(tc.tile_pool(name="sbuf", bufs=1))
        expp = ctx.enter_context(tc.tile_pool(name="expp", bufs=10))
        outp = ctx.enter_context(tc.tile_pool(name="outp", bufs=2))
        # PSUM banks: sc x4 + kv x2 (pt shares tag) + psQ x1 + po x1 = 8
        ps = ctx.enter_context(tc.tile_pool(name="ps", bufs=4, space="PSUM"))
        pq = ctx.enter_context(tc.tile_pool(name="pq", bufs=1, space="PSUM"))
        pop = ctx.enter_context(tc.tile_pool(name="pop", bufs=1, space="PSUM"))

        # ---- const DMAs: first on the SYNC ring (~0.9us ahead of k0).
        # All DMAs on ONE ring transfer strictly FIFO at full aggregate
        # bandwidth; a second active ring (e.g. scalar) interleaves and
        # wrecks completion order, so everything rides sync.
        w_sb = consts.tile([P, DC, 4, DK], BF16, tag="wall")
        w8_sb = consts.tile([P, DC, 3, DK], F8, tag="w8")
        wv_sb = consts.tile([P, DC, DK], BF16, tag="wv")
        bqd_sb = consts.tile([P, 1], F32, tag="bqd")
        if use_dr:
            nc.sync.dma_start(w8_sb[:], w8_d[:])
        else:
            nc.sync.dma_start(w_sb[:], wall_d[:])
        nc.sync.dma_start(bqd_sb[:], bqd_d[:])

        # ---- input stream: one queue, consumption order, depth-3 chain
        kact = [kvp.tile([P, DC, QT], F8, tag="kact", name=f"kact{t}")
                for t in range(NKV)]
        vact = [kvp.tile([P, DC, QT], BF16, tag="vact", name=f"vact{t}")
                for t in range(NKV)]
        qact = [qp.tile([P, DC, QT], F8, tag="qact", name=f"qact{t}")
                for t in range(NQT)]
        in_stream = [
            (qact[0][:], qT_d[0]),
            (kact[0][:], kT_d[0]), (kact[1][:], kT_d[1]),
            (wv_sb[:], wv_d[:]),
            (vact[0][:], vT_d[0]), (vact[1][:], vT_d[1]),
            (qact[1][:], qT_d[1]), (qact[2][:], qT_d[2]),
            (qact[3][:], qT_d[3]),
        ]
        for dst, src_ap in in_stream:
            nc.sync.dma_start(dst, src_ap)

        # ---- persistent SBUF ------------------------------------------
        ident = consts.tile([P, P], BF16)
        make_identity(nc, ident)
        # kt2: strip A (partitions 0:64) = key chunks 0,1 of each kv
        # tile; strip B (64:128) = chunks 2,3. No zero padding.
        kt2 = sbuf.tile([P, NKV, 2 * P], BF16, tag="kt2")
        v_pack = sbuf.tile([P, S_KV // P, VPAD], BF16, tag="v_pack")
        nc.vector.memset(v_pack[:, :, DK:VPAD], 1.0)

        # ---- PE warm-up: dummy matmuls so HAM reaches K=8/8 early -----
        dmy_in = consts.tile([P, QT], BF16, tag="dmy")
        nc.vector.memset(dmy_in[:], 0.0)
        # single accumulation group: back-to-back MMs (no pool-rotation
        # stalls), ~3.4us of continuous PE busy -> HAM locks to K=8/8
        dmy = pop.tile([VW, QT], F32, tag="po", name="dmy")
        for i in range(n_warmup):
            nc.tensor.matmul(dmy[0:DK, :], dmy_in[:, 0:DK], dmy_in[:],
                             start=(i == 0), stop=(i == n_warmup - 1))

        w_q2 = w_sb[:, :, 0:2, :]    # bf16 fallback q weights (duplicated)
        w_k = w_sb[:, :, 2, :]
        w_v = wv_sb

        # ---- helpers ---------------------------------------------------
        kvps = [ps.tile([P, QT], F32, tag="kv", bufs=2, name=f"kv{t}")
                for t in range(NKV)]

        def k_proj(t):
            psK = kvps[t]
            if use_dr:
                for cp in range(DC // 2):
                    nc.tensor.matmul(psK[0:DK, :],
                                     w8_sb[:, 2 * cp:2 * cp + 2, 2, :],
                                     kact[t][:, 2 * cp:2 * cp + 2, :],
                                     start=(cp == 0), stop=(cp == DC // 2 - 1),
                                     perf_mode=DR)
            else:
                for c in range(DC):
                    nc.tensor.matmul(psK[0:DK, :], w_k[:, c, :],
                                     kact[t][:, c, :],
                                     start=(c == 0), stop=(c == DC - 1))
            # kt strips: chunks 0,1 -> partitions 0:64; 2,3 -> 64:128
            nc.scalar.activation(kt2[0:DK, t, :], psK[0:DK, 0:2 * P],
                                 mybir.ActivationFunctionType.Copy)
            nc.scalar.activation(kt2[DK:P, t, :], psK[0:DK, 2 * P:QT],
                                 mybir.ActivationFunctionType.Copy)

        def v_proj(t):
            # v lands in rows 64:128 of the SAME bank as k (col offset 64)
            psV = kvps[t]
            for c in range(DC):
                nc.tensor.matmul(psV[DK:P, :], w_v[:, c, :], vact[t][:, c, :],
                                 start=(c == 0), stop=(c == DC - 1))
            vt_st = outp.tile([DK, QT], BF16, tag="vt_st", name=f"vst{t}")
            nc.scalar.activation(vt_st[:], psV[DK:P, :],
                                 mybir.ActivationFunctionType.Copy)
            pt = ps.tile([P, CPT, DK], BF16, tag="kv", bufs=2,
                         name=f"pvt{t}")
            for ci in range(CPT):
                nc.tensor.transpose(
                    pt[:, ci, :], vt_st[:, ci * P:(ci + 1) * P],
                    ident[0:DK, 0:DK])
            nc.vector.tensor_copy(
                v_pack[:, t * CPT:(t + 1) * CPT, 0:DK], pt[:])

        def q_proj(t):
            psQ = pq.tile([P, QT], F32, tag="psQ", name=f"psQ{t}")
            if use_dr:
                for cp in range(DC // 2):
                    nc.tensor.matmul(
                        psQ[:], w8_sb[:, 2 * cp:2 * cp + 2, 0:2, :],
                        qact[t][:, 2 * cp:2 * cp + 2, :],
                        start=(cp == 0), stop=(cp == DC // 2 - 1),
                        perf_mode=DR)
            else:
                for c in range(DC):
                    nc.tensor.matmul(psQ[:], w_q2[:, c, :, :],
                                     qact[t][:, c, :],
                                     start=(c == 0), stop=(c == DC - 1))
            qt = qp.tile([P, QT], BF16, tag="qt", name=f"qt{t}")
            # one full-width (128-partition) op: half-width ops run at
            # half lane efficiency. Engine alternates per tile.
            if t % 2 == 0:
                nc.scalar.activation(
                    qt[:], psQ[:],
                    mybir.ActivationFunctionType.Identity, bias=bqd_sb[:])
            else:
                nc.vector.tensor_scalar(
                    qt[:], psQ[:], bqd_sb[:], None, mybir.AluOpType.add)
            return qt

        def scores_attnv(t, qt, defer_po=False, next_hook=None):
            # attn@v runs one slot behind scores so the PE never stalls
            # on exp latency (ACT/DVE take ~0.7us per chunk). With
            # defer_po, ALL attn@v matmuls come after the scores (tile 0:
            # v_pack isn't ready yet and interleaved po's would block the
            # PE FIFO).
            po = pop.tile([VW, QT], F32, tag="po", name=f"po{t}")
            slot = 0
            pend = []
            emitted = [0]

            def emit_po(kc, e):
                nc.tensor.matmul(
                    po[:], v_pack[:, kc, 0:VW], e[:],
                    start=(emitted[0] == 0),
                    stop=(emitted[0] == S_KV // P - 1))
                emitted[0] += 1

            for tk in range(NKV):
                for p in range(2):
                    # concurrent row-tiled pair (K=64 strips)
                    kcA = 4 * tk + p          # global key chunk, strip A
                    kcB = 4 * tk + 2 + p      # strip B
                    scA = ps.tile([P, QT], F32, tag="sc",
                                  name=f"scA{t}{tk}{p}")
                    scB = ps.tile([P, QT], F32, tag="sc",
                                  name=f"scB{t}{tk}{p}")
                    nc.tensor.matmul(scA[:], kt2[0:DK, tk, p * P:(p + 1) * P],
                                     qt[0:DK, :], start=True, stop=True,
                                     tile_position=(0, 0))
                    nc.tensor.matmul(scB[:], kt2[DK:P, tk, p * P:(p + 1) * P],
                                     qt[DK:P, :], start=True, stop=True,
                                     tile_position=(DK, 0))
                    if not defer_po:
                        for kc, e in pend:
                            emit_po(kc, e)
                        pend = []
                    for kc, sc in ((kcA, scA), (kcB, scB)):
                        e = expp.tile([P, QT], BF16, tag="exp",
                                      name=f"e{t}{kc}")
                        a = SCHRAU_A * (ESCALE if use_dr else 1.0)
                        if t == NQT - 1 and tk == NKV - 1:
                            # tail latency: halve across both engines
                            nc.scalar.activation(
                                e[:, 0:QT // 2], sc[:, 0:QT // 2],
                                mybir.ActivationFunctionType.Exp,
                                scale=ESCALE if use_dr else 1.0)
                            nc.vector.tensor_scalar(
                                e[:, QT // 2:QT].bitcast(I16),
                                sc[:, QT // 2:QT], a, SCHRAU_B,
                                mybir.AluOpType.mult, mybir.AluOpType.add)
                            slot += 1
                            pend.append((kc, e))
                            continue
                        eng = exp_pattern[slot % len(exp_pattern)]
                        slot += 1
                        if eng == "a":
                            nc.scalar.activation(
                                e[:], sc[:],
                                mybir.ActivationFunctionType.Exp,
                                scale=ESCALE if use_dr else 1.0)
                        else:
                            nc.vector.tensor_scalar(
                                e[:].bitcast(I16), sc[:], a, SCHRAU_B,
                                mybir.AluOpType.mult, mybir.AluOpType.add)
                        pend.append((kc, e))
            if defer_po:
                v_proj(0)
                v_proj(1)
            res_next = next_hook() if next_hook is not None else None
            for kc, e in pend:
                emit_po(kc, e)
            return po, res_next

        def store(t, po):
            tq = slice(t * QT, (t + 1) * QT)
            o = outp.tile([VW, QT], BF16, tag="out", name=f"o{t}")
            if t == NQT - 1:
                # final tile: halve the copy latency across both engines
                nc.scalar.activation(o[:, 0:QT // 2], po[:, 0:QT // 2],
                                     mybir.ActivationFunctionType.Copy)
                nc.vector.tensor_copy(o[:, QT // 2:QT], po[:, QT // 2:QT])
            elif t % 2 == 0:
                nc.scalar.activation(o[:], po[:],
                                     mybir.ActivationFunctionType.Copy)
            else:
                nc.vector.tensor_copy(o[:], po[:])
            nc.sync.dma_start(out_d[:, tq], o[:])

        # ---- pipeline ---------------------------------------------------
        # Tile 0: scores (and exps) run before the v projections in the
        # PE program, its attn@v matmuls after them (defer_po). Each
        # tile's q projection for t+1 is emitted inside tile t's po tail
        # (fills the PE bubble while the last exps land).
        qt0 = q_proj(0)
        k_proj(0)
        k_proj(1)
        qt_next = [None]

        def hook(t):
            def h():
                return q_proj(t + 1) if t + 1 < NQT else None
            return h

        po0, qt1 = scores_attnv(0, qt0, defer_po=True, next_hook=hook(0))
        store(0, po0)
        qt = qt1
        for t in range(1, NQT):
            po, qt_n = scores_attnv(t, qt, next_hook=hook(t))
            store(t, po)
            qt = qt_n

    nc.compile()
    return nc


_CACHED = {}


def _get_program():
    key = ("v53", USE_DR)
    if key not in _CACHED:
        _CACHED[key] = build_program()
    return _CACHED[key]


def _tileify(a2d, ntiles, dtype):
    # [D, ntiles*QT] -> [ntiles, P, DC, QT] tile-major blocks
    dd = a2d.shape[0]
    return np.ascontiguousarray(
        a2d.reshape(P, dd // P, ntiles, QT).transpose(2, 0, 1, 3)
    ).astype(dtype)


def make_in_maps(query, key, value, Wq, bq, Wk, bk, Wv, bv, use_dr=USE_DR):
    # bk is unused: it only shifts scores by a per-query constant, which
    # cancels in softmax. bv is added on the host in assemble_output.
    q = np.asarray(query, dtype=np.float32)
    k = np.asarray(key, dtype=np.float32)
    v = np.asarray(value, dtype=np.float32)
    wqf = np.asarray(Wq, np.float32).reshape(P, DC, DK)
    wkf = np.asarray(Wk, np.float32).reshape(P, DC, DK)
    wvf = np.asarray(Wv, np.float32).reshape(P, DC, DK)
    if use_dr:
        # fp8 weights scaled x16; bias matches the x16 q scale
        bqd = np.tile((np.asarray(bq, np.float32) * WSCALE).reshape(-1, 1),
                      (2, 1))
        w8 = np.stack([wqf * WSCALE, wqf * WSCALE, wkf * WSCALE],
                      axis=2).astype(F8NP)
    else:
        bqd = np.tile((np.asarray(bq, np.float32) * 0.125).reshape(-1, 1),
                      (2, 1))
        w8 = np.zeros((P, DC, 3, DK), F8NP)
    wq_bf = wqf * (1.0 if use_dr else 0.125)
    wall = np.stack([wq_bf, wq_bf, wkf, wvf], axis=2).astype(BF)
    consts = {
        "Wall": np.ascontiguousarray(wall),
        "W8": np.ascontiguousarray(w8),
        "Wv16": np.ascontiguousarray(wvf.astype(BF)),
        "bqd": np.ascontiguousarray(bqd),
    }
    in_maps = []
    for i in range(N_CORES):
        b, kvh = divmod(i, 2)
        sl = slice(kvh * S_KV, (kvh + 1) * S_KV)
        in_maps.append({
            "qT": _tileify(q[b].T, NQT, F8NP),
            "kT": _tileify(np.ascontiguousarray(k[b, sl].T), NKV, F8NP),
            "vT": _tileify(np.ascontiguousarray(v[b, sl].T), NKV, BF),
            **consts,
        })
    return in_maps


def assemble_output(results, bv):
    bvf = np.asarray(bv, np.float32).reshape(1, DK)
    out = np.empty((B, S, DK), np.float32)
    for b in range(B):
        r0 = results[2 * b]["out"].astype(np.float32)
        r1 = results[2 * b + 1]["out"].astype(np.float32)
        num = r0[0:DK] + r1[0:DK]
        den = r0[DK:VW] + r1[DK:VW]
        out[b] = (num / den).T + bvf
    return out


def kernel(query, key, value, Wq, bq, Wk, bk, Wv, bv, **run_kwargs):
    nc = _get_program()
    in_maps = make_in_maps(query, key, value, Wq, bq, Wk, bk, Wv, bv)
    res = run_bass_kernel_spmd(nc, in_maps, core_ids=list(range(N_CORES)),
                               **run_kwargs)
    out = assemble_output(res.results, bv)
    if run_kwargs.get("trace"):
        kernel.last_result = res
    return out


# revision 24
# speedup vs baseline: 1.0438x; 1.0438x over previous
"""AttentionHead kernel for 8 TRN2 NeuronCores — v5.7 (streaming + fp8 DR).

Reference computation (B=4, S=2048, D=1024, dk=dv=64):
    q = query @ Wq + bq ; k = key @ Wk + bk ; v = value @ Wv + bv
    out = softmax(q @ k.T / 8) @ v

Sharding: core i handles batch b = i//2 and KEY/VALUE half kvh = i%2:
it attends ALL 2048 queries of its batch against its 1024 keys, producing
a partial softmax numerator [64, 2048] and denominator [1, 2048]. Since
max-subtraction is skipped (scores std ~0.33), the host combines halves
by adding numerators and denominators, then divides — zero d2d traffic.

Key design points (lineage 149 -> 98/67 -> 61 -> this):
  * Tile-major activation layouts ([tile, P, DC, 512] blocks): each tile
    is one 128-descriptor HWDGE DMA. fp8 q/k ride RAW (the PE accepts
    fp8 moving operands); no gpsimd SWDGE anywhere.
  * ALL DMAs (consts + inputs) ride ONE HWDGE ring (sync) in
    consumption order: same-ring transfers are strictly FIFO at full
    aggregate bandwidth (measured), so tensors complete in stream order
    (q0, k0, k1, v0, v1, q1..q3) with zero chaining overhead. A second
    active ring would interleave and wreck the order; completion-
    chaining via deps costs 2-3us dead time per link. Both measured
    worse.
  * q and k projections run as fp8 DoubleRow matmuls (2 D-chunks per
    instruction, 2x): weights ship as fp8 scaled by 16 (entries +-0.5,
    inside e4m3 normal range; unscaled they'd be subnormal), activations
    are the raw fp8 inputs. The resulting 256x score scale is folded
    into the exp: ACT Exp uses scale=1/2048, the Schraudolph multiplier
    absorbs the same factor. 16*bq rides as bias on the qt copy.
  * v projection stays bf16 (quantizing v costs ~2.4% end-to-end).
  * Scores are row-tiled: kt2 holds two 64-contraction strips (key
    chunks on partitions 0:64 / 64:128); two K=64 matmuls at
    tile_position (0,0)/(64,0) overlap on the PE. The q projection
    lands duplicated on both partition halves via host-duplicated Wq
    columns (free: output partitions don't cost cycles).
  * attn@v is software-pipelined one slot behind scores so the PE never
    stalls on exp latency; exp splits ACT (exact, scaled Exp) / DVE
    (Schraudolph: i16 = round(x*A + 16248.5) bitcast bf16 ~ e^x).
    q_proj(t+1) is emitted into tile t's attn@v tail (fills the PE
    bubble); an 8-MM single-accumulation warmup locks HAM to K=8/8
    before the first projection. PSUM: sc x4 + kv x2 (k rows 0:64 via
    DoubleRow, v rows 64:128 via col offset, pt shares) + psQ x1 +
    po x1 = 8 banks.
  * Softmax denominator via a ones-column appended to v (row 64 of po).
  * Per-tile output copy (split ACT/DVE halves) + per-tile DMA.
"""

import os
import sys

if "/opt/trn_rl_repo" not in sys.path:
    sys.path.insert(0, "/opt/trn_rl_repo")

import numpy as np
import ml_dtypes

import concourse.bass as bass
import concourse.mybir as mybir
import concourse.tile as tile
from concourse import bacc
from concourse.bass_utils import run_bass_kernel_spmd
from concourse.masks import make_identity
from concourse.tile_rust import add_dep_helper

N_CORES = 8
B, S, D, DK = 4, 2048, 1024, 64
S_KV = S // 2           # per-core key/value rows
P = 128
DC = D // P             # 8 contraction chunks
QT = 512                # tile width (PSUM bank: 512 f32)
NQT = S // QT           # 4 query tiles
NKV = S_KV // QT        # 2 kv tiles
CPT = QT // P           # key chunks per kv tile (4)
VW = DK + 1             # v plus ones-column
VPAD = 66               # v_pack row stride (VW padded to 4B multiple)
F32 = mybir.dt.float32
BF16 = mybir.dt.bfloat16
F8 = mybir.dt.float8e4
I16 = mybir.dt.int16
BF = ml_dtypes.bfloat16
F8NP = ml_dtypes.float8_e4m3
DR = mybir.MatmulPerfMode.DoubleRow

# Schraudolph exp for bf16: bitcast(int16(round(x * 128/ln2 + b))) ~ e^x
SCHRAU_A = 128.0 * 1.4426950408889634
SCHRAU_B = 16248.5
WSCALE = 16.0           # fp8 weight scale (q/k); descaled inside exp
ESCALE = 1.0 / (8.0 * WSCALE * WSCALE)   # exp(scores_hw * ESCALE)

N_WARMUP = int(os.environ.get("BASS_ATTN_WARMUP", "11"))
# exp engine per key-chunk slot, cycled: a=ACT (exact), v=DVE (Schraudolph)
EXP_PATTERN = os.environ.get("BASS_ATTN_EXP_PATTERN", "avavaavv")
USE_DR = os.environ.get("BASS_ATTN_DR", "1") == "1"   # fp8 DoubleRow q/k proj
CHAIN_DEPTH = int(os.environ.get("BASS_ATTN_CHAIN", "3"))


def build_program(n_warmup=N_WARMUP, exp_pattern=EXP_PATTERN, use_dr=USE_DR):
    nc = bacc.Bacc("TRN2", target_bir_lowering=False, debug=False,
                   num_devices=N_CORES)

    qT_d = nc.dram_tensor("qT", [NQT, P, DC, QT], F8, kind="ExternalInput")
    kT_d = nc.dram_tensor("kT", [NKV, P, DC, QT], F8, kind="ExternalInput")
    vT_d = nc.dram_tensor("vT", [NKV, P, DC, QT], BF16, kind="ExternalInput")
    # Wall (bf16 path): [P, DC, 4, DK] = [Wq | Wq | Wk | Wv]
    # W8 (DoubleRow path): [P, DC, 3, DK] fp8 = [16Wq | 16Wq | 16Wk]
    wall_d = nc.dram_tensor("Wall", [P, DC, 4, DK], BF16,
                            kind="ExternalInput")
    w8_d = nc.dram_tensor("W8", [P, DC, 3, DK], F8, kind="ExternalInput")
    wv_d = nc.dram_tensor("Wv16", [P, DC, DK], BF16, kind="ExternalInput")
    bqd_d = nc.dram_tensor("bqd", [P, 1], F32, kind="ExternalInput")
    # rows 0:64 = partial attn@v numerator, row 64 = partial softmax
    # denominator; the host combines kv-halves, divides, adds bv.
    out_d = nc.dram_tensor("out", [VW, S], BF16, kind="ExternalOutput")

    from contextlib import ExitStack

    with tile.TileContext(nc) as tc, ExitStack() as ctx:
        consts = ctx.enter_context(tc.tile_pool(name="consts", bufs=1))
        kvp = ctx.enter_context(tc.tile_pool(name="kvp", bufs=2))
        qp = ctx.enter_context(tc.tile_pool(name="qp", bufs=4))
        sbuf = ctx.enter_context(tc.tile_pool(name="sbuf", bufs=1))
        expp = ctx.enter_context(tc.tile_pool(name="expp", bufs=10))
        outp = ctx.enter_context(tc.tile_pool(name="outp", bufs=2))
        # PSUM banks: sc x4 + kv x2 (pt shares tag) + psQ x1 + po x1 = 8
        ps = ctx.enter_context(tc.tile_pool(name="ps", bufs=4, space="PSUM"))
        pq = ctx.enter_context(tc.tile_pool(name="pq", bufs=1, space="PSUM"))
        pop = ctx.enter_context(tc.tile_pool(name="pop", bufs=1, space="PSUM"))

        # ---- const DMAs: first on the SYNC ring (~0.9us ahead of k0).
        # All DMAs on ONE ring transfer strictly FIFO at full aggregate
        # bandwidth; a second active ring (e.g. scalar) interleaves and
        # wrecks completion order, so everything rides sync.
        w_sb = consts.tile([P, DC, 4, DK], BF16, tag="wall")
        w8_sb = consts.tile([P, DC, 3, DK], F8, tag="w8")
        wv_sb = consts.tile([P, DC, DK], BF16, tag="wv")
        bqd_sb = consts.tile([P, 1], F32, tag="bqd")
        if use_dr:
            nc.sync.dma_start(w8_sb[:], w8_d[:])
        else:
            nc.sync.dma_start(w_sb[:], wall_d[:])
        nc.sync.dma_start(bqd_sb[:], bqd_d[:])

        # ---- input stream: one queue, consumption order, depth-3 chain
        kact = [kvp.tile([P, DC, QT], F8, tag="kact", name=f"kact{t}")
                for t in range(NKV)]
        vact = [kvp.tile([P, DC, QT], BF16, tag="vact", name=f"vact{t}")
                for t in range(NKV)]
        qact = [qp.tile([P, DC, QT], F8, tag="qact", name=f"qact{t}")
                for t in range(NQT)]
        in_stream = [
            (qact[0][:], qT_d[0]),
            (kact[0][:], kT_d[0]), (kact[1][:], kT_d[1]),
            (wv_sb[:], wv_d[:]),
            (vact[0][:], vT_d[0]), (vact[1][:], vT_d[1]),
            (qact[1][:], qT_d[1]), (qact[2][:], qT_d[2]),
            (qact[3][:], qT_d[3]),
        ]
        for dst, src_ap in in_stream:
            nc.sync.dma_start(dst, src_ap)

        # ---- persistent SBUF ------------------------------------------
        ident = consts.tile([P, P], BF16)
        make_identity(nc, ident)
        # kt2: strip A (partitions 0:64) = key chunks 0,1 of each kv
        # tile; strip B (64:128) = chunks 2,3. No zero padding.
        kt2 = sbuf.tile([P, NKV, 2 * P], BF16, tag="kt2")
        v_pack = sbuf.tile([P, S_KV // P, VPAD], BF16, tag="v_pack")
        nc.vector.memset(v_pack[:, :, DK:VPAD], 1.0)

        # ---- PE warm-up: dummy matmuls so HAM reaches K=8/8 early -----
        dmy_in = consts.tile([P, QT], BF16, tag="dmy")
        nc.vector.memset(dmy_in[:], 0.0)
        # single accumulation group: back-to-back MMs (no pool-rotation
        # stalls), ~3.4us of continuous PE busy -> HAM locks to K=8/8
        dmy = pop.tile([VW, QT], F32, tag="po", name="dmy")
        for i in range(n_warmup):
            nc.tensor.matmul(dmy[0:DK, :], dmy_in[:, 0:DK], dmy_in[:],
                             start=(i == 0), stop=(i == n_warmup - 1))

        w_q2 = w_sb[:, :, 0:2, :]    # bf16 fallback q weights (duplicated)
        w_k = w_sb[:, :, 2, :]
        w_v = wv_sb

        # ---- helpers ---------------------------------------------------
        kvps = [ps.tile([P, QT], F32, tag="kv", bufs=2, name=f"kv{t}")
                for t in range(NKV)]

        def k_proj(t):
            psK = kvps[t]
            if use_dr:
                for cp in range(DC // 2):
                    nc.tensor.matmul(psK[0:DK, :],
                                     w8_sb[:, 2 * cp:2 * cp + 2, 2, :],
                                     kact[t][:, 2 * cp:2 * cp + 2, :],
                                     start=(cp == 0), stop=(cp == DC // 2 - 1),
                                     perf_mode=DR)
            else:
                for c in range(DC):
                    nc.tensor.matmul(psK[0:DK, :], w_k[:, c, :],
                                     kact[t][:, c, :],
                                     start=(c == 0), stop=(c == DC - 1))
            # kt strips: chunks 0,1 -> partitions 0:64; 2,3 -> 64:128
            nc.scalar.activation(kt2[0:DK, t, :], psK[0:DK, 0:2 * P],
                                 mybir.ActivationFunctionType.Copy)
            nc.scalar.activation(kt2[DK:P, t, :], psK[0:DK, 2 * P:QT],
                                 mybir.ActivationFunctionType.Copy)

        def v_proj(t):
            # v lands in rows 64:128 of the SAME bank as k (col offset 64)
            psV = kvps[t]
            for c in range(DC):
                nc.tensor.matmul(psV[DK:P, :], w_v[:, c, :], vact[t][:, c, :],
                                 start=(c == 0), stop=(c == DC - 1))
            vt_st = outp.tile([DK, QT], BF16, tag="vt_st", name=f"vst{t}")
            nc.scalar.activation(vt_st[:], psV[DK:P, :],
                                 mybir.ActivationFunctionType.Copy)
            pt = ps.tile([P, CPT, DK], BF16, tag="kv", bufs=2,
                         name=f"pvt{t}")
            for ci in range(CPT):
                nc.tensor.transpose(
                    pt[:, ci, :], vt_st[:, ci * P:(ci + 1) * P],
                    ident[0:DK, 0:DK])
            nc.vector.tensor_copy(
                v_pack[:, t * CPT:(t + 1) * CPT, 0:DK], pt[:])

        def q_proj(t):
            psQ = pq.tile([P, QT], F32, tag="psQ", name=f"psQ{t}")
            if use_dr:
                for cp in range(DC // 2):
                    nc.tensor.matmul(
                        psQ[:], w8_sb[:, 2 * cp:2 * cp + 2, 0:2, :],
                        qact[t][:, 2 * cp:2 * cp + 2, :],
                        start=(cp == 0), stop=(cp == DC // 2 - 1),
                        perf_mode=DR)
            else:
                for c in range(DC):
                    nc.tensor.matmul(psQ[:], w_q2[:, c, :, :],
                                     qact[t][:, c, :],
                                     start=(c == 0), stop=(c == DC - 1))
            qt = qp.tile([P, QT], BF16, tag="qt", name=f"qt{t}")
            # one full-width (128-partition) op: half-width ops run at
            # half lane efficiency. Engine alternates per tile.
            if t % 2 == 0:
                nc.scalar.activation(
                    qt[:], psQ[:],
                    mybir.ActivationFunctionType.Identity, bias=bqd_sb[:])
            else:
                nc.vector.tensor_scalar(
                    qt[:], psQ[:], bqd_sb[:], None, mybir.AluOpType.add)
            return qt

        def scores_attnv(t, qt, defer_po=False, next_hook=None):
            # attn@v runs one slot behind scores so the PE never stalls
            # on exp latency (ACT/DVE take ~0.7us per chunk). With
            # defer_po, ALL attn@v matmuls come after the scores (tile 0:
            # v_pack isn't ready yet and interleaved po's would block the
            # PE FIFO).
            po = pop.tile([VW, QT], F32, tag="po", name=f"po{t}")
            slot = 0
            pend = []
            emitted = [0]

            def emit_po(kc, e):
                nc.tensor.matmul(
                    po[:], v_pack[:, kc, 0:VW], e[:],
                    start=(emitted[0] == 0),
                    stop=(emitted[0] == S_KV // P - 1))
                emitted[0] += 1

            for tk in range(NKV):
                for p in range(2):
                    # concurrent row-tiled pair (K=64 strips)
                    kcA = 4 * tk + p          # global key chunk, strip A
                    kcB = 4 * tk + 2 + p      # strip B
                    scA = ps.tile([P, QT], F32, tag="sc",
                                  name=f"scA{t}{tk}{p}")
                    scB = ps.tile([P, QT], F32, tag="sc",
                                  name=f"scB{t}{tk}{p}")
                    nc.tensor.matmul(scA[:], kt2[0:DK, tk, p * P:(p + 1) * P],
                                     qt[0:DK, :], start=True, stop=True,
                                     tile_position=(0, 0))
                    nc.tensor.matmul(scB[:], kt2[DK:P, tk, p * P:(p + 1) * P],
                                     qt[DK:P, :], start=True, stop=True,
                                     tile_position=(DK, 0))
                    if not defer_po:
                        for kc, e in pend:
                            emit_po(kc, e)
                        pend = []
                    for kc, sc in ((kcA, scA), (kcB, scB)):
                        e = expp.tile([P, QT], BF16, tag="exp",
                                      name=f"e{t}{kc}")
                        eng = exp_pattern[slot % len(exp_pattern)]
                        slot += 1
                        if eng == "a":
                            nc.scalar.activation(
                                e[:], sc[:],
                                mybir.ActivationFunctionType.Exp,
                                scale=ESCALE if use_dr else 1.0)
                        else:
                            a = SCHRAU_A * (ESCALE if use_dr else 1.0)
                            nc.vector.tensor_scalar(
                                e[:].bitcast(I16), sc[:], a, SCHRAU_B,
                                mybir.AluOpType.mult, mybir.AluOpType.add)
                        pend.append((kc, e))
            if defer_po:
                v_proj(0)
                v_proj(1)
            res_next = next_hook() if next_hook is not None else None
            for kc, e in pend:
                emit_po(kc, e)
            return po, res_next

        def store(t, po):
            tq = slice(t * QT, (t + 1) * QT)
            o = outp.tile([VW, QT], BF16, tag="out", name=f"o{t}")
            if t % 2 == 0:
                nc.vector.tensor_copy(o[:], po[:])
            else:
                nc.scalar.activation(o[:], po[:],
                                     mybir.ActivationFunctionType.Copy)
            nc.sync.dma_start(out_d[:, tq], o[:])

        # ---- pipeline ---------------------------------------------------
        # Tile 0: scores (and exps) run before the v projections in the
        # PE program, its attn@v matmuls after them (defer_po). Each
        # tile's q projection for t+1 is emitted inside tile t's po tail
        # (fills the PE bubble while the last exps land).
        qt0 = q_proj(0)
        k_proj(0)
        k_proj(1)
        qt_next = [None]

        def hook(t):
            def h():
                return q_proj(t + 1) if t + 1 < NQT else None
            return h

        po0, qt1 = scores_attnv(0, qt0, defer_po=True, next_hook=hook(0))
        store(0, po0)
        qt = qt1
        for t in range(1, NQT):
            po, qt_n = scores_attnv(t, qt, next_hook=hook(t))
            store(t, po)
            qt = qt_n

    nc.compile()
    return nc


_CACHED = {}


def _get_program():
    key = ("v53", USE_DR)
    if key not in _CACHED:
        _CACHED[key] = build_program()
    return _CACHED[key]


def _tileify(a2d, ntiles, dtype):
    # [D, ntiles*QT] -> [ntiles, P, DC, QT] tile-major blocks
    dd = a2d.shape[0]
    return np.ascontiguousarray(
        a2d.reshape(P, dd // P, ntiles, QT).transpose(2, 0, 1, 3)
    ).astype(dtype)


def make_in_maps(query, key, value, Wq, bq, Wk, bk, Wv, bv, use_dr=USE_DR):
    # bk is unused: it only shifts scores by a per-query constant, which
    # cancels in softmax. bv is added on the host in assemble_output.
    q = np.asarray(query, dtype=np.float32)
    k = np.asarray(key, dtype=np.float32)
    v = np.asarray(value, dtype=np.float32)
    wqf = np.asarray(Wq, np.float32).reshape(P, DC, DK)
    wkf = np.asarray(Wk, np.float32).reshape(P, DC, DK)
    wvf = np.asarray(Wv, np.float32).reshape(P, DC, DK)
    if use_dr:
        # fp8 weights scaled x16; bias matches the x16 q scale
        bqd = np.tile((np.asarray(bq, np.float32) * WSCALE).reshape(-1, 1),
                      (2, 1))
        w8 = np.stack([wqf * WSCALE, wqf * WSCALE, wkf * WSCALE],
                      axis=2).astype(F8NP)
    else:
        bqd = np.tile((np.asarray(bq, np.float32) * 0.125).reshape(-1, 1),
                      (2, 1))
        w8 = np.zeros((P, DC, 3, DK), F8NP)
    wq_bf = wqf * (1.0 if use_dr else 0.125)
    wall = np.stack([wq_bf, wq_bf, wkf, wvf], axis=2).astype(BF)
    consts = {
        "Wall": np.ascontiguousarray(wall),
        "W8": np.ascontiguousarray(w8),
        "Wv16": np.ascontiguousarray(wvf.astype(BF)),
        "bqd": np.ascontiguousarray(bqd),
    }
    in_maps = []
    for i in range(N_CORES):
        b, kvh = divmod(i, 2)
        sl = slice(kvh * S_KV, (kvh + 1) * S_KV)
        in_maps.append({
            "qT": _tileify(q[b].T, NQT, F8NP),
            "kT": _tileify(np.ascontiguousarray(k[b, sl].T), NKV, F8NP),
            "vT": _tileify(np.ascontiguousarray(v[b, sl].T), NKV, BF),
            **consts,
        })
    return in_maps


def assemble_output(results, bv):
    bvf = np.asarray(bv, np.float32).reshape(1, DK)
    out = np.empty((B, S, DK), np.float32)
    for b in range(B):
        r0 = results[2 * b]["out"].astype(np.float32)
        r1 = results[2 * b + 1]["out"].astype(np.float32)
        num = r0[0:DK] + r1[0:DK]
        den = r0[DK:VW] + r1[DK:VW]
        out[b] = (num / den).T + bvf
    return out


def kernel(query, key, value, Wq, bq, Wk, bk, Wv, bv, **run_kwargs):
    nc = _get_program()
    in_maps = make_in_maps(query, key, value, Wq, bq, Wk, bk, Wv, bv)
    res = run_bass_kernel_spmd(nc, in_maps, core_ids=list(range(N_CORES)),
                               **run_kwargs)
    out = assemble_output(res.results, bv)
    if run_kwargs.get("trace"):
        kernel.last_result = res
    return out
